# revision 1
# baseline (speedup 1.0000x reference)
"""Real spherical harmonics Y_lm (l<=8) on 8 TRN2 NeuronCores.

Data-parallel over the 1M points. Per core: 125k points padded to
128*990; partition-major layout so each partition owns a contiguous
row range of the [N, 81] output -> output DMA is 128 large contiguous
runs per chunk.

All normalization constants are folded into the Legendre recurrences
(scaled P~ = ctil(l,m) * P_l^m), so each three-term recurrence is two
fused scalar_tensor_tensor ops and each output column is a single
tensor_tensor multiply with sin(m phi) / cos(m phi) from the ACT LUT.
"""

import math
import sys

sys.path.insert(0, "/opt/trn_rl_repo")

import numpy as np

import concourse.bass as bass
import concourse.mybir as mybir
from concourse.tile import TileContext
from concourse.bass_utils import run_bass_kernel_spmd

F32 = mybir.dt.float32
AF = mybir.ActivationFunctionType
OP = mybir.AluOpType

N_TOTAL = 1_000_000
NCORES = 8
PER = N_TOTAL // NCORES      # 125000 real points per core
P = 128                      # SBUF partitions
LPP = 990                    # points per partition (padded)
PADN = P * LPP               # 126720 padded points per core
FD = 198                     # free-dim chunk size
NCHUNK = LPP // FD           # 5
LMAX = 8
NCOL = (LMAX + 1) ** 2       # 81


# ACT Sin LUT domain is [-pi, pi]; we feed it t - pi with t = arg mod 2pi,
# which yields -sin(arg). The global -1 is folded into ctil(l,m) for m>=1
# (it cancels in every recurrence ratio, which are all within-m or
# diag-chain ctil ratios two m apart).
TWO_PI_LO = float(np.nextafter(np.float32(2 * math.pi), np.float32(0.0)))
PI_LO = float(np.nextafter(np.float32(math.pi), np.float32(0.0)))


def _ctil():
    """ctil(l,m) * P_l^m(x) * ang(m, phi) = output column, with the
    reference's 1/sqrt(2) for m=0 folded in. m>=1 entries are negated
    to absorb the -sin from the range-reduced LUT trig."""
    c = {}
    for l in range(LMAX + 1):
        c[(l, 0)] = math.sqrt((2 * l + 1) / (4 * math.pi))
        for m in range(1, l + 1):
            c[(l, m)] = -((-1.0) ** m) * math.sqrt(2.0) * math.sqrt(
                (2 * l + 1) / (4 * math.pi)
                * math.factorial(l - m) / math.factorial(l + m)
            )
    return c


def _lrec_ab(l, m, C):
    """P~(l,m) = a*x*P~(l-1,m) + b*P~(l-2,m)."""
    alpha = (2 * l - 1) / (l - m)
    beta = -(l + m - 1) / (l - m)
    a = alpha * C[(l, m)] / C[(l - 1, m)]
    b = beta * C[(l, m)] / C[(l - 2, m)]
    return a, b


def build_nc(lpp=LPP, fd=FD, fds=None):
    # fds: per-chunk free-dim sizes (sum == lpp). A smaller final chunk
    # shrinks the exposed tail DMA after the last compute finishes.
    if fds is None:
        fds = [fd] * (lpp // fd)
    assert sum(fds) == lpp
    padn = P * lpp
    C = _ctil()
    nc = bass.Bass()
    ct = nc.declare_dram_parameter("cos_theta", [padn], F32, isOutput=False)
    ph = nc.declare_dram_parameter("phi", [padn], F32, isOutput=False)
    out = nc.declare_dram_parameter("out", [padn * NCOL], F32, isOutput=True)

    ctv = ct[:].rearrange("(p f) -> p f", p=P)
    phv = ph[:].rearrange("(p f) -> p f", p=P)
    outv = out[:].rearrange("(p f) -> p f", p=P)

    with TileContext(nc) as tc:
        with (
            tc.tile_pool(name="res", bufs=1) as res_pool,
            tc.tile_pool(name="work", bufs=2) as work_pool,
            tc.tile_pool(name="obuf", bufs=2) as o_pool,
        ):
            xt = res_pool.tile([P, lpp], F32)
            pt = res_pool.tile([P, lpp], F32)
            # Sin needs AP biases (-pi and -pi/2); memset a const tile
            # inside the Tile context so deps are tracked (no barrier).
            cbias = res_pool.tile([P, 2], F32)
            nc.gpsimd.memset(cbias[:, 0:1], -PI_LO)
            nc.gpsimd.memset(cbias[:, 1:2], -PI_LO / 2)
            bias_negpi = cbias[:, 0:1]
            bias_neghalfpi = cbias[:, 1:2]

            off = 0
            for c, fd in enumerate(fds):
                sl = slice(off, off + fd)
                ocolbase = off * NCOL
                off += fd
                nc.sync.dma_start(out=xt[:, sl], in_=ctv[:, sl])
                nc.sync.dma_start(out=pt[:, sl], in_=phv[:, sl])
                x = xt[:, sl]
                f = pt[:, sl]

                w = work_pool.tile([P, fd * 38], F32)

                def W(i):
                    return w[:, i * fd:(i + 1) * fd]

                def WP(i):
                    # two adjacent fd slices as [P, 2, fd] (pair OUTER:
                    # the DVE then streams long stride runs, same as the
                    # unpaired ops, instead of a 2-element inner zigzag)
                    return w[:, i * fd:(i + 2) * fd].rearrange(
                        "p (k f) -> p k f", k=2
                    )

                def WF(i):
                    # two adjacent fd slices flat [P, 2*fd] (for all-
                    # contiguous paired ops, cheapest AP form)
                    return w[:, i * fd:(i + 2) * fd]

                x2, s, b, b2 = W(0), W(1), W(2), W(3)
                s2a, s2P = W(4), WP(4)          # s2 doubled
                TP = WP(6)                      # T pair
                twoC1a, twoC1b = W(8), W(9)
                twoC1F = WF(8)
                xxa, xxb, xxP = W(10), W(11), WP(10)
                DP = [WP(12 + 2 * k) for k in range(4)]   # diag pair ring
                DS = [W(12 + 2 * k) for k in range(4)]    # first slot of each
                uF = WF(20)
                SIN = [None] + [W(22 + 2 * (m - 1)) for m in range(1, 9)]
                COS = [None] + [W(23 + 2 * (m - 1)) for m in range(1, 9)]
                TRIGP = [None] + [WP(22 + 2 * (m - 1)) for m in range(1, 9)]
                TRIGF = [None] + [WF(22 + 2 * (m - 1)) for m in range(1, 9)]

                O = o_pool.tile([P, fd * NCOL], F32)
                O3 = O.rearrange("p (f c) -> p f c", c=NCOL)
                O3c = O.rearrange("p (f c) -> p c f", c=NCOL)

                def ocol(j):
                    return O3[:, :, j]

                def opair(j0, dm):
                    # columns j0 and j0+dm as [P, 2, fd] (pair outer)
                    return O3c[:, j0:j0 + dm + 1:dm, :]

                # ---- column (0,0) first: absorbs the WAR-vs-DMA wait on
                # this O slot in a single-dependency DVE op. O must only
                # ever be written by DVE (cross-engine writers would need
                # a second wait slot the TT ISA struct doesn't have).
                nc.vector.tensor_scalar(
                    ocol(0), x, 0.0, C[(0, 0)], OP.mult, OP.add
                )

                # ---- ACT: all single-source affine/transcendental work.
                # s = sqrt(1-x^2); doubled copies feed the paired DVE ops.
                nc.scalar.activation(x2, x, AF.Square)
                nc.scalar.activation(s, x2, AF.Sqrt, bias=1.0, scale=-1.0)
                nc.scalar.activation(s2a, x2, AF.Copy, scale=-1.0, bias=1.0)
                nc.scalar.activation(W(5), x2, AF.Copy, scale=-1.0, bias=1.0)
                nc.scalar.activation(xxa, x, AF.Copy)
                nc.scalar.activation(xxb, x, AF.Copy)
                # trig seeds: SIN[m]/COS[m] hold -sin/-cos(m phi) (the -1
                # lives in ctil). ACT Sin domain is [-pi,pi]:
                # SIN[1] = Sin(phi-pi) = -sin(phi); b = Sin(phi/2 - pi/2)
                # = -cos(phi/2); COS[1] = 1-2b^2 = -cos(phi);
                # twoC1 = 4b^2-2 = 2cos(phi). Chebyshev:
                # X'_m = twoC1*X'_{m-1} - X'_{m-2}, S'_0 = 0, C'_0 = -1.
                nc.scalar.activation(SIN[1], f, AF.Sin, bias=bias_negpi)
                nc.scalar.activation(
                    b, f, AF.Sin, scale=0.5, bias=bias_neghalfpi
                )
                nc.scalar.activation(b2, b, AF.Square)
                nc.scalar.activation(twoC1a, b2, AF.Copy, scale=4.0, bias=-2.0)
                nc.scalar.activation(twoC1b, b2, AF.Copy, scale=4.0, bias=-2.0)
                nc.scalar.activation(COS[1], b2, AF.Copy, scale=-2.0, bias=1.0)
                # diagonal seeds (doubled): P~(1,1) = -ctil(1,1)*s,
                # P~(2,2) = 3*ctil(2,2)*s^2
                nc.scalar.activation(DS[1], s, AF.Copy, scale=-C[(1, 1)])
                nc.scalar.activation(W(15), s, AF.Copy, scale=-C[(1, 1)])
                nc.scalar.activation(DS[2], s2a, AF.Copy, scale=3.0 * C[(2, 2)])
                nc.scalar.activation(W(17), s2a, AF.Copy, scale=3.0 * C[(2, 2)])

                # ---- DVE trig recurrence (paired sin|cos, flat APs) ----
                nc.vector.tensor_tensor(TRIGF[2], twoC1F, TRIGF[1], OP.mult)
                nc.vector.tensor_scalar(COS[2], COS[2], 1.0, None, OP.add)
                for m in range(3, 9):
                    nc.vector.tensor_tensor(uF, twoC1F, TRIGF[m - 1], OP.mult)
                    nc.vector.scalar_tensor_tensor(
                        TRIGF[m], TRIGF[m - 2], -1.0, uF, OP.mult, OP.add
                    )

                # ---- m = 0 chain: P~(l,0) is directly column l*l+l ----
                T0 = W(6)
                nc.vector.tensor_scalar(ocol(2), x, C[(1, 0)], None, OP.mult)
                a, bb = _lrec_ab(2, 0, C)
                nc.vector.scalar_tensor_tensor(T0, ocol(2), a, x, OP.mult, OP.mult)
                nc.vector.tensor_scalar(
                    ocol(6), T0, bb * C[(0, 0)], None, OP.add
                )
                for l in range(3, 9):
                    a, bb = _lrec_ab(l, 0, C)
                    nc.vector.scalar_tensor_tensor(
                        T0, ocol((l - 1) * l), a, x, OP.mult, OP.mult
                    )
                    nc.vector.scalar_tensor_tensor(
                        ocol(l * l + l), ocol((l - 2) * (l - 1)), bb, T0,
                        OP.mult, OP.add,
                    )

                # ---- m >= 1: columns satisfy the l-recurrence directly
                # (it is linear, the trig factor distributes), so all
                # work runs on +-m column PAIRS in one instruction. ----
                for m in range(1, 9):
                    if m >= 3:
                        Am = (2 * m - 1) * (2 * m - 3) * C[(m, m)] / C[(m - 2, m - 2)]
                        nc.vector.scalar_tensor_tensor(
                            DP[m & 3], DP[(m - 2) & 3], Am, s2P,
                            OP.mult, OP.mult,
                        )
                    jb = m * m + m
                    nc.vector.tensor_tensor(
                        opair(jb - m, 2 * m), DP[m & 3], TRIGP[m], OP.mult
                    )
                    if m <= 7:
                        Em = (2 * m + 1) * C[(m + 1, m)] / C[(m, m)]
                        j1 = (m + 1) * (m + 2)
                        nc.vector.scalar_tensor_tensor(
                            opair(j1 - m, 2 * m), opair(jb - m, 2 * m), Em,
                            xxP, OP.mult, OP.mult,
                        )
                        for l in range(m + 2, 9):
                            a, bb = _lrec_ab(l, m, C)
                            nc.vector.scalar_tensor_tensor(
                                TP, opair((l - 1) * l - m, 2 * m), a, xxP,
                                OP.mult, OP.mult,
                            )
                            nc.vector.scalar_tensor_tensor(
                                opair(l * l + l - m, 2 * m),
                                opair((l - 2) * (l - 1) - m, 2 * m), bb, TP,
                                OP.mult, OP.add,
                            )

                nc.sync.dma_start(
                    out=outv[:, ocolbase:ocolbase + fd * NCOL],
                    in_=O[:, :],
                )
    _legalize_waits(nc)
    return nc


_TPB_COMPUTE = (
    mybir.InstTensorTensor,
    mybir.InstTensorScalarPtr,
    mybir.InstActivation,
    mybir.InstTensorCopy,
    mybir.InstTensorReduce,
    mybir.InstMemset,
)


def _legalize_waits(nc):
    """TPB compute ISA structs encode a single sync-wait slot; Tile can
    emit 2+ waits on one instruction (walrus then fails with 'Too many
    sync wait commands'). Hoist all but one wait onto NoOps in front."""
    f = nc.m.functions[0]
    for b in f.blocks:
        insts = b.instructions
        idx = 0
        while idx < len(insts):
            i = insts[idx]
            si = i.sync_info
            if si is not None and len(si.on_wait) > 1:
                waits = list(si.on_wait)
                for wextra in waits[:-1]:
                    nop = mybir.InstEventSemaphore(
                        name=nc.get_next_instruction_name(), ins=[], outs=[]
                    )
                    nop.engine = i.engine
                    nop.sync_info = mybir.SyncInfo(
                        on_wait=[wextra], on_update=[]
                    )
                    nc.register_instruction(nop)
                    insts.insert(idx, nop)
                    idx += 1
                si.on_wait = [waits[-1]]
            idx += 1


_NC_CACHE = None


# Uneven chunks: clock-normalized DVE busy is identical to uniform
# (290.5 us, verified in-trace), but the smaller final chunk cuts the
# exposed tail DMA from 8.2 MB (23 us) to 5.6 MB (15.5 us).
# [216x4, 126] no longer fits SBUF; 214 is the limit.
FDS = [214, 214, 214, 214, 134]


def _get_nc():
    global _NC_CACHE
    if _NC_CACHE is None:
        _NC_CACHE = build_nc(fds=FDS)
    return _NC_CACHE


# NOTE: identical NEFFs measure either ~324 us or ~384 us depending on
# which physical cores the process lands on (DVE/ACT at 0.96 vs 0.8
# GHz — visible as an exact 1.2x scale on engine-busy in the trace).
# In-process warm-up bursts do not change the state; it is placement/
# machine-side, so no kernel-side mitigation exists.
def _run(cos_theta, phi, trace=False, **kw):
    cos_theta = np.ascontiguousarray(np.asarray(cos_theta), dtype=np.float32)
    phi = np.ascontiguousarray(np.asarray(phi), dtype=np.float32)
    assert cos_theta.shape == (N_TOTAL,) and phi.shape == (N_TOTAL,)
    in_maps = []
    for i in range(NCORES):
        c = np.zeros(PADN, np.float32)
        p_ = np.zeros(PADN, np.float32)
        c[:PER] = cos_theta[i * PER:(i + 1) * PER]
        p_[:PER] = phi[i * PER:(i + 1) * PER]
        in_maps.append({"cos_theta": c, "phi": p_})
    res = run_bass_kernel_spmd(
        _get_nc(), in_maps, core_ids=list(range(NCORES)), trace=trace, **kw
    )
    outs = [
        np.asarray(r["out"]).reshape(PADN, NCOL)[:PER] for r in res.results
    ]
    return np.concatenate(outs, axis=0), res


def kernel(cos_theta, phi):
    out, _ = _run(cos_theta, phi)
    return out



# revision 4
# speedup vs baseline: 1.6922x; 1.6922x over previous
"""Real spherical harmonics Y_lm (l<=8) on 8 TRN2 NeuronCores.

Data-parallel over the 1M points; per core 125k points padded to
128*977. All device compute and the output are fp16 (tolerance is
2e-2; fp16 end-to-end lands ~2e-3): DVE tensor_tensor runs at 2x and
tensor_scalar at 4x with packed 2-byte operands, while
scalar_tensor_tensor gets no speedup — so the three-term Legendre
recurrences are rescaled per (l,m) chain (P' = g*P~) to force the
P(l-2) coefficient to exactly -1, turning each step into one STT (the
a*x*P product) plus one fp16 TT subtract. Output tile is
column-major per partition so every column write is packed; the HBM
output is [81, n] and the host transposes, applies the per-column 1/g
scales (and odd-m sign flips from the sqrt-based diagonal seed), and
casts to f32.
"""

import math
import sys

sys.path.insert(0, "/opt/trn_rl_repo")

import numpy as np

import concourse.bass as bass
import concourse.mybir as mybir
from concourse.ap import AP
from concourse.tile import TileContext
from concourse.bass_utils import run_bass_kernel_spmd

F32 = mybir.dt.float32
F16 = mybir.dt.float16
AF = mybir.ActivationFunctionType
OP = mybir.AluOpType

N_TOTAL = 1_000_000
NCORES = 8
PER = N_TOTAL // NCORES      # 125000 real points per core
P = 128                      # SBUF partitions
LPP = 977                    # points per partition (128*977 = 125056)
PADN = P * LPP
LMAX = 8
NCOL = (LMAX + 1) ** 2       # 81

# ACT Sin LUT domain is [-pi, pi]; we feed t - pi, yielding -sin(t).
# The global -1 for m>=1 is folded into ctil (it cancels through every
# recurrence, which only ever mixes same-m or m+-2 ctil ratios).
TWO_PI_LO = float(np.nextafter(np.float32(2 * math.pi), np.float32(0.0)))
PI_LO = float(np.nextafter(np.float32(math.pi), np.float32(0.0)))


def _ctil():
    c = {}
    for l in range(LMAX + 1):
        c[(l, 0)] = math.sqrt((2 * l + 1) / (4 * math.pi))
        for m in range(1, l + 1):
            c[(l, m)] = -((-1.0) ** m) * math.sqrt(2.0) * math.sqrt(
                (2 * l + 1) / (4 * math.pi)
                * math.factorial(l - m) / math.factorial(l + m)
            )
    return c


def _coeffs():
    """xa[(l,m)]: scalar on x in  P'(l) = (P'(l-1) * xa) * x - P'(l-2);
    g[(l,m)]: P'(l,m) = g * P~(l,m) (seeds g=1)."""
    C = _ctil()
    xa, g = {}, {}
    for m in range(0, LMAX):
        g[(m, m)] = 1.0
        g[(m + 1, m)] = 1.0
        for l in range(m + 2, LMAX + 1):
            alpha = (2 * l - 1) / (l - m)
            beta = -(l + m - 1) / (l - m)
            a2 = alpha * C[(l, m)] / C[(l - 1, m)]
            b2 = beta * C[(l, m)] / C[(l - 2, m)]
            g[(l, m)] = -g[(l - 2, m)] / b2
            xa[(l, m)] = a2 * g[(l, m)] / g[(l - 1, m)]
    g[(8, 8)] = 1.0
    return xa, g, C


def _colscale():
    """Host-side per-column multiplier: 1/g, with a -1 for odd-m columns
    (the device's diagonal seed is +C11*s from a Sqrt, i.e. -P~(1,1))."""
    _, g, _ = _coeffs()
    cs = np.ones(NCOL, np.float32)
    for l in range(1, 9):
        cs[l * l + l] = 1.0 / g[(l, 0)]
    for m in range(1, 9):
        flip = -1.0 if (m % 2) else 1.0
        for l in range(m, 9):
            j = l * l + l
            cs[j - m] = flip / g[(l, m)]
            cs[j + m] = flip / g[(l, m)]
    return cs


def _bpair(a, k=2):
    """[P, f] AP -> [P, k, f] with stride-0 middle dim (free broadcast)."""
    d = a.ap
    assert len(d) == 2
    return AP(a.tensor, a.offset, [list(d[0]), [0, k], list(d[1])])


def build_nc(fds):
    assert sum(fds) == LPP
    fdmax = max(fds)
    xa, g, C = _coeffs()
    C00, C10, C11 = C[(0, 0)], C[(1, 0)], C[(1, 1)]

    nc = bass.Bass()
    ct = nc.declare_dram_parameter("cos_theta", [PADN], F32, isOutput=False)
    ph = nc.declare_dram_parameter("phi", [PADN], F32, isOutput=False)
    out = nc.declare_dram_parameter("out", [PADN * NCOL], F16, isOutput=True)

    ctv = ct[:].rearrange("(p f) -> p f", p=P)
    phv = ph[:].rearrange("(p f) -> p f", p=P)
    # HBM layout [c, p, f]: per partition, each column chunk is one
    # contiguous fd*2-byte run.
    outv = out[:].rearrange("(c p f) -> p c f", c=NCOL, p=P)

    # work-tile f16 slice indices
    S_XH = 0
    S_S2 = 1
    S_2C = 2      # twoC1 pair (2 slices)
    S_U = 4       # u pair (2 slices)
    S_TRIG = 6    # 8 pairs m=1..8 (16 slices)
    S_D = 22      # diag ring (4 slices)
    S_TM = 26     # shared inner scratch
    S_T0 = 27     # m0 chain scratch
    S_PB = 28     # P'(l,m) m=1..7, l=m+1..8 (28 slices)
    pidx = {}
    _n = S_PB
    for m in range(1, 8):
        for l in range(m + 1, 9):
            pidx[(l, m)] = _n
            _n += 1
    NSL = _n  # 56

    with TileContext(nc) as tc:
        with (
            tc.tile_pool(name="res", bufs=1) as res_pool,
            tc.tile_pool(name="work", bufs=2) as work_pool,
            tc.tile_pool(name="obuf", bufs=2) as o_pool,
        ):
            xt = res_pool.tile([P, LPP], F32)
            pt = res_pool.tile([P, LPP], F32)
            cbias = res_pool.tile([P, 3], F32)
            nc.gpsimd.memset(cbias[:, 0:1], -PI_LO)
            nc.gpsimd.memset(cbias[:, 1:2], -PI_LO / 2)
            nc.gpsimd.memset(cbias[:, 2:3], C11 * C11)
            bias_negpi = cbias[:, 0:1]
            bias_neghalfpi = cbias[:, 1:2]
            bias_c11sq = cbias[:, 2:3]
            # trig T'_0 = (0, -1) constant pair
            t0c = res_pool.tile([P, 2 * fdmax], F16)
            nc.gpsimd.memset(t0c[:, 0:fdmax], 0.0)
            nc.gpsimd.memset(t0c[:, fdmax:2 * fdmax], -1.0)

            off = 0
            for fd in fds:
                sl = slice(off, off + fd)
                ooff = off
                off += fd
                nc.sync.dma_start(out=xt[:, sl], in_=ctv[:, sl])
                nc.sync.dma_start(out=pt[:, sl], in_=phv[:, sl])
                x = xt[:, sl]
                f = pt[:, sl]

                w = work_pool.tile([P, NSL * fd], F16)
                x2f = work_pool.tile([P, fd], F32)
                bbf = work_pool.tile([P, fd], F32)
                b2f = work_pool.tile([P, fd], F32)

                def W(i):
                    return w[:, i * fd:(i + 1) * fd]

                def WP(i):
                    return w[:, i * fd:(i + 2) * fd].rearrange(
                        "p (k f) -> p k f", k=2
                    )

                T0P = t0c[:, :].rearrange("p (k f) -> p k f", k=2)[:, :, 0:fd]

                def TRIGP(m):
                    return WP(S_TRIG + 2 * (m - 1))

                xh, s2h = W(S_XH), W(S_S2)

                def D(m):
                    return W(S_D + (m & 3))

                O = o_pool.tile([P, NCOL * fd], F16)
                O3 = O.rearrange("p (c f) -> p c f", c=NCOL)

                def ocol(j):
                    return O3[:, j, :]

                def opair(l, m):
                    j = l * l + l
                    return O3[:, j - m:j + m + 1:2 * m, :]

                # ---- ACT: transcendental + affine seeds ----
                nc.scalar.activation(x2f, x, AF.Square)
                nc.scalar.activation(xh, x, AF.Copy)
                nc.scalar.activation(s2h, x2f, AF.Copy, scale=-1.0, bias=1.0)
                # +C11*s == -P~(1,1): odd-m sign fixed on host
                nc.scalar.activation(
                    D(1), x2f, AF.Sqrt, scale=-(C11 * C11), bias=bias_c11sq
                )
                nc.scalar.activation(
                    D(2), x2f, AF.Copy,
                    scale=-3.0 * C[(2, 2)], bias=3.0 * C[(2, 2)],
                )
                SINP = TRIGP(1)
                nc.scalar.activation(
                    W(S_TRIG), f, AF.Sin, bias=bias_negpi
                )  # -sin(phi)
                nc.scalar.activation(
                    bbf, f, AF.Sin, scale=0.5, bias=bias_neghalfpi
                )  # -cos(phi/2)
                nc.scalar.activation(b2f, bbf, AF.Square)
                nc.scalar.activation(W(S_2C), b2f, AF.Copy, scale=4.0, bias=-2.0)
                nc.scalar.activation(
                    W(S_2C + 1), b2f, AF.Copy, scale=4.0, bias=-2.0
                )
                nc.scalar.activation(
                    W(S_TRIG + 1), b2f, AF.Copy, scale=-2.0, bias=1.0
                )  # -cos(phi)

                # ---- O constants ----
                nc.gpsimd.memset(ocol(0), C00)
                nc.vector.tensor_scalar(ocol(2), xh, C10, None, OP.mult)

                # ---- trig: T'_m = twoC1*T'_{m-1} - T'_{m-2} (fp16 TT) ----
                for m in range(2, 9):
                    prev2 = T0P if m == 2 else TRIGP(m - 2)
                    nc.vector.tensor_tensor(
                        WP(S_U), WP(S_2C), TRIGP(m - 1), OP.mult
                    )
                    nc.vector.tensor_tensor(
                        TRIGP(m), WP(S_U), prev2, OP.subtract
                    )

                # ---- m = 0 chain: columns l*l+l directly ----
                T0 = W(S_T0)
                nc.vector.scalar_tensor_tensor(
                    T0, ocol(2), xa[(2, 0)], xh, OP.mult, OP.mult
                )
                nc.vector.tensor_scalar(ocol(6), T0, C00, None, OP.subtract)
                for l in range(3, 9):
                    nc.vector.scalar_tensor_tensor(
                        T0, ocol((l - 1) * l), xa[(l, 0)], xh, OP.mult, OP.mult
                    )
                    nc.vector.tensor_tensor(
                        ocol(l * l + l), T0, ocol((l - 2) * (l - 1)),
                        OP.subtract,
                    )

                # ---- m >= 1 ----
                TM = W(S_TM)
                for m in range(1, 9):
                    if m >= 3:
                        Am = (2 * m - 1) * (2 * m - 3) * C[(m, m)] / C[(m - 2, m - 2)]
                        nc.vector.scalar_tensor_tensor(
                            D(m), D(m - 2), Am, s2h, OP.mult, OP.mult
                        )
                    nc.vector.tensor_tensor(
                        opair(m, m), _bpair(D(m)), TRIGP(m), OP.mult
                    )
                    if m <= 7:
                        Em = (2 * m + 1) * C[(m + 1, m)] / C[(m, m)]
                        Pf = W(pidx[(m + 1, m)])
                        nc.vector.scalar_tensor_tensor(
                            Pf, D(m), Em, xh, OP.mult, OP.mult
                        )
                        nc.vector.tensor_tensor(
                            opair(m + 1, m), _bpair(Pf), TRIGP(m), OP.mult
                        )
                        for l in range(m + 2, 9):
                            Pc = W(pidx[(l, m)])
                            nc.vector.scalar_tensor_tensor(
                                TM, W(pidx[(l - 1, m)]), xa[(l, m)], xh,
                                OP.mult, OP.mult,
                            )
                            nc.vector.tensor_tensor(
                                Pc, TM,
                                D(m) if l == m + 2 else W(pidx[(l - 2, m)]),
                                OP.subtract,
                            )
                            nc.vector.tensor_tensor(
                                opair(l, m), _bpair(Pc), TRIGP(m), OP.mult
                            )

                nc.sync.dma_start(
                    out=outv[:, :, ooff:ooff + fd], in_=O3[:, :, :]
                )
    _legalize_waits(nc)
    return nc


def _legalize_waits(nc):
    """TPB compute ISA structs encode a single sync-wait slot; Tile can
    emit 2+ waits on one instruction. Hoist extras onto NoOps."""
    f = nc.m.functions[0]
    for b in f.blocks:
        insts = b.instructions
        idx = 0
        while idx < len(insts):
            i = insts[idx]
            si = i.sync_info
            if si is not None and len(si.on_wait) > 1:
                waits = list(si.on_wait)
                for wextra in waits[:-1]:
                    nop = mybir.InstEventSemaphore(
                        name=nc.get_next_instruction_name(), ins=[], outs=[]
                    )
                    nop.engine = i.engine
                    nop.sync_info = mybir.SyncInfo(
                        on_wait=[wextra], on_update=[]
                    )
                    nc.register_instruction(nop)
                    insts.insert(idx, nop)
                    idx += 1
                si.on_wait = [waits[-1]]
            idx += 1


_NC_CACHE = None

# Smaller final chunk shrinks the exposed tail DMA.
FDS = [345, 345, 287]


def _get_nc():
    global _NC_CACHE
    if _NC_CACHE is None:
        _NC_CACHE = build_nc(FDS)
    return _NC_CACHE


def _run(cos_theta, phi, trace=False, **kw):
    cos_theta = np.ascontiguousarray(np.asarray(cos_theta), dtype=np.float32)
    phi = np.ascontiguousarray(np.asarray(phi), dtype=np.float32)
    assert cos_theta.shape == (N_TOTAL,) and phi.shape == (N_TOTAL,)
    in_maps = []
    for i in range(NCORES):
        c = np.zeros(PADN, np.float32)
        p_ = np.zeros(PADN, np.float32)
        c[:PER] = cos_theta[i * PER:(i + 1) * PER]
        p_[:PER] = phi[i * PER:(i + 1) * PER]
        in_maps.append({"cos_theta": c, "phi": p_})
    res = run_bass_kernel_spmd(
        _get_nc(), in_maps, core_ids=list(range(NCORES)), trace=trace, **kw
    )
    cs = _colscale()
    outs = []
    for r in res.results:
        a = np.asarray(r["out"]).reshape(NCOL, PADN).T[:PER]  # [PER, 81] f16
        outs.append(a.astype(np.float32) * cs[None, :])
    return np.concatenate(outs, axis=0), res


def kernel(cos_theta, phi):
    out, _ = _run(cos_theta, phi)
    return out


# revision 5
# speedup vs baseline: 1.9611x; 1.1589x over previous
"""Real spherical harmonics Y_lm (l<=8) on 8 TRN2 NeuronCores.

Data-parallel over the 1M points; per core 125k points padded to
128*977. All device compute and the output are fp16 (tolerance is
2e-2; fp16 end-to-end lands ~2e-3): DVE tensor_tensor runs at 2x and
tensor_scalar at 4x with packed 2-byte operands, while
scalar_tensor_tensor gets no speedup — so the three-term Legendre
recurrences are rescaled per (l,m) chain (P' = g*P~) to force the
P(l-2) coefficient to exactly -1, turning each step into one STT (the
a*x*P product) plus one fp16 TT subtract.

The output tile is column-major per partition with columns PERMUTED
into m-grouped blocks: each m-block's +-m emits are two wide TTs over
contiguous columns (trig broadcast via a stride-0 AP), and the output
DMA is striped into 4 column ranges per chunk that fire as soon as
their m-blocks finish, alternating between the SP and ACT HWDGE queue
groups. The host inverts the permutation, applies per-column 1/g
scales (and odd-m sign flips from the sqrt-based diagonal seed), and
casts to f32.
"""

import math
import sys

sys.path.insert(0, "/opt/trn_rl_repo")

import numpy as np

import concourse.bass as bass
import concourse.mybir as mybir
from concourse.ap import AP
from concourse.tile import TileContext
from concourse.bass_utils import run_bass_kernel_spmd

F32 = mybir.dt.float32
F16 = mybir.dt.float16
AF = mybir.ActivationFunctionType
OP = mybir.AluOpType

N_TOTAL = 1_000_000
NCORES = 8
PER = N_TOTAL // NCORES      # 125000 real points per core
P = 128                      # SBUF partitions
LPP = 977                    # points per partition (128*977 = 125056)
PADN = P * LPP
LMAX = 8
NCOL = (LMAX + 1) ** 2       # 81

PI_LO = float(np.nextafter(np.float32(math.pi), np.float32(0.0)))

# device column order: m0 block (l=0..8), then per m: sin(l=m..8),
# cos(l=m..8)
MBASE = {}
_b = 9
for _m in range(1, 9):
    MBASE[_m] = _b
    _b += 2 * (9 - _m)
assert _b == NCOL


def _devcol(l, m_signed):
    """true (l, m) -> device column index."""
    if m_signed == 0:
        return l
    m = abs(m_signed)
    base = MBASE[m] + (0 if m_signed < 0 else 9 - m)
    return base + (l - m)


def _ctil():
    c = {}
    for l in range(LMAX + 1):
        c[(l, 0)] = math.sqrt((2 * l + 1) / (4 * math.pi))
        for m in range(1, l + 1):
            c[(l, m)] = -((-1.0) ** m) * math.sqrt(2.0) * math.sqrt(
                (2 * l + 1) / (4 * math.pi)
                * math.factorial(l - m) / math.factorial(l + m)
            )
    return c


def _coeffs():
    """xa[(l,m)]: scalar on x in  P'(l) = (P'(l-1) * xa) * x - P'(l-2);
    g[(l,m)]: P'(l,m) = g * P~(l,m) (seeds g=1)."""
    C = _ctil()
    xa, g = {}, {}
    for m in range(0, LMAX):
        g[(m, m)] = 1.0
        g[(m + 1, m)] = 1.0
        for l in range(m + 2, LMAX + 1):
            alpha = (2 * l - 1) / (l - m)
            beta = -(l + m - 1) / (l - m)
            a2 = alpha * C[(l, m)] / C[(l - 1, m)]
            b2 = beta * C[(l, m)] / C[(l - 2, m)]
            g[(l, m)] = -g[(l - 2, m)] / b2
            xa[(l, m)] = a2 * g[(l, m)] / g[(l - 1, m)]
    g[(8, 8)] = 1.0
    return xa, g, C


def _host_maps():
    """(order, scale): full_out[:, j_true] = dev[:, order[j_true]] * scale[j_true].

    scale = 1/g with a -1 for odd-m columns (the device diagonal seed is
    +C11*s from a Sqrt, i.e. -P~(1,1))."""
    _, g, _ = _coeffs()
    order = np.zeros(NCOL, np.int64)
    scale = np.ones(NCOL, np.float32)
    for l in range(0, 9):
        for ms in range(-l, l + 1):
            j = l * l + l + ms
            order[j] = _devcol(l, ms)
            m = abs(ms)
            flip = -1.0 if (m % 2) else 1.0
            scale[j] = (flip if m else 1.0) / g[(l, m)]
    return order, scale


def _bpair(a, k=2):
    """[P, f] AP -> [P, k, f] with stride-0 middle dim (free broadcast)."""
    d = a.ap
    assert len(d) == 2
    return AP(a.tensor, a.offset, [list(d[0]), [0, k], list(d[1])])


def build_nc(fds):
    assert sum(fds) == LPP
    fdmax = max(fds)
    xa, g, C = _coeffs()
    C00, C10, C11 = C[(0, 0)], C[(1, 0)], C[(1, 1)]

    nc = bass.Bass()
    ct = nc.declare_dram_parameter("cos_theta", [PADN], F32, isOutput=False)
    ph = nc.declare_dram_parameter("phi", [PADN], F32, isOutput=False)
    out = nc.declare_dram_parameter("out", [PADN * NCOL], F16, isOutput=True)

    ctv = ct[:].rearrange("(p f) -> p f", p=P)
    phv = ph[:].rearrange("(p f) -> p f", p=P)
    # HBM layout [c, p, f]: per partition, each column chunk is one
    # contiguous fd*2-byte run.
    outv = out[:].rearrange("(c p f) -> p c f", c=NCOL, p=P)

    # work-tile f16 slice indices
    S_XH = 0
    S_S2 = 1
    S_2C = 2      # twoC1 pair (2 slices)
    S_U = 4       # u pair (2 slices)
    S_TRIG = 6    # 8 pairs m=1..8 (16 slices: sin, cos adjacent)
    S_D = 22      # diag ring (4 slices)
    S_TM = 26     # shared inner scratch
    S_T0 = 27     # m0 chain scratch
    S_PB = 28     # P'(l,m) m=1..7, l=m+1..8 (28 slices, chain-contiguous)
    pidx = {}
    _n = S_PB
    for m in range(1, 8):
        for l in range(m + 1, 9):
            pidx[(l, m)] = _n
            _n += 1
    NSL = _n  # 56

    # output DMA stripes: (device col range, fires after m-block)
    STRIPES = [
        (0, MBASE[2], 1),            # m0 + m1 cols, after m=1
        (MBASE[2], MBASE[4], 3),     # m2 + m3, after m=3
        (MBASE[4], MBASE[6], 5),     # m4 + m5, after m=5
        (MBASE[6], NCOL, 8),         # m6..m8, after m=8
    ]

    with TileContext(nc) as tc:
        with (
            tc.tile_pool(name="res", bufs=1) as res_pool,
            tc.tile_pool(name="work", bufs=2) as work_pool,
            tc.tile_pool(name="obuf", bufs=2) as o_pool,
        ):
            xt = res_pool.tile([P, LPP], F32)
            pt = res_pool.tile([P, LPP], F32)
            cbias = res_pool.tile([P, 3], F32)
            nc.gpsimd.memset(cbias[:, 0:1], -PI_LO)
            nc.gpsimd.memset(cbias[:, 1:2], -PI_LO / 2)
            nc.gpsimd.memset(cbias[:, 2:3], C11 * C11)
            bias_negpi = cbias[:, 0:1]
            bias_neghalfpi = cbias[:, 1:2]
            bias_c11sq = cbias[:, 2:3]
            # trig T'_0 = (0, -1) constant pair
            t0c = res_pool.tile([P, 2 * fdmax], F16)
            nc.gpsimd.memset(t0c[:, 0:fdmax], 0.0)
            nc.gpsimd.memset(t0c[:, fdmax:2 * fdmax], -1.0)

            off = 0
            dma_rr = 0
            for fd in fds:
                sl = slice(off, off + fd)
                ooff = off
                off += fd
                nc.sync.dma_start(out=xt[:, sl], in_=ctv[:, sl])
                nc.scalar.dma_start(out=pt[:, sl], in_=phv[:, sl])
                x = xt[:, sl]
                f = pt[:, sl]

                w = work_pool.tile([P, NSL * fd], F16)
                x2f = work_pool.tile([P, fd], F32)
                bbf = work_pool.tile([P, fd], F32)
                b2f = work_pool.tile([P, fd], F32)

                def W(i):
                    return w[:, i * fd:(i + 1) * fd]

                def WB(i, k):
                    return w[:, i * fd:(i + k) * fd].rearrange(
                        "p (k f) -> p k f", k=k
                    )

                T0P = t0c[:, :].rearrange("p (k f) -> p k f", k=2)[:, :, 0:fd]

                def TRIGP(m):
                    return WB(S_TRIG + 2 * (m - 1), 2)

                xh, s2h = W(S_XH), W(S_S2)

                def D(m):
                    return W(S_D + (m & 3))

                O = o_pool.tile([P, NCOL * fd], F16)
                O3 = O.rearrange("p (c f) -> p c f", c=NCOL)

                def ocol(j):
                    return O3[:, j, :]

                # ---- ACT: transcendental + affine seeds ----
                nc.scalar.activation(x2f, x, AF.Square)
                nc.scalar.activation(xh, x, AF.Copy)
                nc.scalar.activation(s2h, x2f, AF.Copy, scale=-1.0, bias=1.0)
                # +C11*s == -P~(1,1): odd-m sign fixed on host
                nc.scalar.activation(
                    D(1), x2f, AF.Sqrt, scale=-(C11 * C11), bias=bias_c11sq
                )
                nc.scalar.activation(
                    D(2), x2f, AF.Copy,
                    scale=-3.0 * C[(2, 2)], bias=3.0 * C[(2, 2)],
                )
                nc.scalar.activation(
                    W(S_TRIG), f, AF.Sin, bias=bias_negpi
                )  # -sin(phi)
                nc.scalar.activation(
                    bbf, f, AF.Sin, scale=0.5, bias=bias_neghalfpi
                )  # -cos(phi/2)
                nc.scalar.activation(b2f, bbf, AF.Square)
                nc.scalar.activation(W(S_2C), b2f, AF.Copy, scale=4.0, bias=-2.0)
                nc.scalar.activation(
                    W(S_2C + 1), b2f, AF.Copy, scale=4.0, bias=-2.0
                )
                nc.scalar.activation(
                    W(S_TRIG + 1), b2f, AF.Copy, scale=-2.0, bias=1.0
                )  # -cos(phi)

                # ---- O constants ----
                nc.gpsimd.memset(ocol(0), C00)
                nc.vector.tensor_scalar(ocol(1), xh, C10, None, OP.mult)

                # ---- trig: T'_m = twoC1*T'_{m-1} - T'_{m-2} (fp16 TT) ----
                for m in range(2, 9):
                    prev2 = T0P if m == 2 else TRIGP(m - 2)
                    nc.vector.tensor_tensor(
                        WB(S_U, 2), WB(S_2C, 2), TRIGP(m - 1), OP.mult
                    )
                    nc.vector.tensor_tensor(
                        TRIGP(m), WB(S_U, 2), prev2, OP.subtract
                    )

                # ---- m = 0 chain: device columns 0..8 directly ----
                T0 = W(S_T0)
                nc.vector.scalar_tensor_tensor(
                    T0, ocol(1), xa[(2, 0)], xh, OP.mult, OP.mult
                )
                nc.vector.tensor_scalar(ocol(2), T0, C00, None, OP.subtract)
                for l in range(3, 9):
                    nc.vector.scalar_tensor_tensor(
                        T0, ocol(l - 1), xa[(l, 0)], xh, OP.mult, OP.mult
                    )
                    nc.vector.tensor_tensor(
                        ocol(l), T0, ocol(l - 2), OP.subtract
                    )

                # ---- m >= 1 blocks + striped output DMA ----
                TM = W(S_TM)
                stripe_i = 0
                for m in range(1, 9):
                    if m >= 3:
                        Am = (2 * m - 1) * (2 * m - 3) * C[(m, m)] / C[(m - 2, m - 2)]
                        nc.vector.scalar_tensor_tensor(
                            D(m), D(m - 2), Am, s2h, OP.mult, OP.mult
                        )
                    if m <= 7:
                        Em = (2 * m + 1) * C[(m + 1, m)] / C[(m, m)]
                        nc.vector.scalar_tensor_tensor(
                            W(pidx[(m + 1, m)]), D(m), Em, xh, OP.mult, OP.mult
                        )
                        for l in range(m + 2, 9):
                            nc.vector.scalar_tensor_tensor(
                                TM, W(pidx[(l - 1, m)]), xa[(l, m)], xh,
                                OP.mult, OP.mult,
                            )
                            nc.vector.tensor_tensor(
                                W(pidx[(l, m)]), TM,
                                D(m) if l == m + 2 else W(pidx[(l - 2, m)]),
                                OP.subtract,
                            )
                    # emits: diagonal pair, then sin/cos blocks l=m+1..8
                    base = MBASE[m]
                    k = 9 - m
                    nc.vector.tensor_tensor(
                        O3[:, base:base + k + 1:k, :], _bpair(D(m)),
                        TRIGP(m), OP.mult,
                    )
                    if m <= 7:
                        pb = WB(pidx[(m + 1, m)], 8 - m)
                        nc.vector.tensor_tensor(
                            O3[:, base + 1:base + k, :], pb,
                            _bpair(W(S_TRIG + 2 * (m - 1)), 8 - m), OP.mult,
                        )
                        nc.vector.tensor_tensor(
                            O3[:, base + k + 1:base + 2 * k, :], pb,
                            _bpair(W(S_TRIG + 2 * m - 1), 8 - m), OP.mult,
                        )
                    while stripe_i < len(STRIPES) and STRIPES[stripe_i][2] == m:
                        c0, c1, _ = STRIPES[stripe_i]
                        eng = nc.sync if (dma_rr % 2 == 0) else nc.scalar
                        eng.dma_start(
                            out=outv[:, c0:c1, ooff:ooff + fd],
                            in_=O3[:, c0:c1, :],
                        )
                        dma_rr += 1
                        stripe_i += 1
    _legalize_waits(nc)
    return nc


def _legalize_waits(nc):
    """TPB compute ISA structs encode a single sync-wait slot; Tile can
    emit 2+ waits on one instruction. Hoist extras onto NoOps."""
    f = nc.m.functions[0]
    for b in f.blocks:
        insts = b.instructions
        idx = 0
        while idx < len(insts):
            i = insts[idx]
            si = i.sync_info
            if si is not None and len(si.on_wait) > 1:
                waits = list(si.on_wait)
                for wextra in waits[:-1]:
                    nop = mybir.InstEventSemaphore(
                        name=nc.get_next_instruction_name(), ins=[], outs=[]
                    )
                    nop.engine = i.engine
                    nop.sync_info = mybir.SyncInfo(
                        on_wait=[wextra], on_update=[]
                    )
                    nc.register_instruction(nop)
                    insts.insert(idx, nop)
                    idx += 1
                si.on_wait = [waits[-1]]
            idx += 1


_NC_CACHE = None

# Smaller final chunk shrinks the exposed tail DMA.
FDS = [345, 345, 287]


def _get_nc():
    global _NC_CACHE
    if _NC_CACHE is None:
        _NC_CACHE = build_nc(FDS)
    return _NC_CACHE


def _run(cos_theta, phi, trace=False, **kw):
    cos_theta = np.ascontiguousarray(np.asarray(cos_theta), dtype=np.float32)
    phi = np.ascontiguousarray(np.asarray(phi), dtype=np.float32)
    assert cos_theta.shape == (N_TOTAL,) and phi.shape == (N_TOTAL,)
    in_maps = []
    for i in range(NCORES):
        c = np.zeros(PADN, np.float32)
        p_ = np.zeros(PADN, np.float32)
        c[:PER] = cos_theta[i * PER:(i + 1) * PER]
        p_[:PER] = phi[i * PER:(i + 1) * PER]
        in_maps.append({"cos_theta": c, "phi": p_})
    res = run_bass_kernel_spmd(
        _get_nc(), in_maps, core_ids=list(range(NCORES)), trace=trace, **kw
    )
    order, scale = _host_maps()
    outs = []
    for r in res.results:
        a = np.asarray(r["out"]).reshape(NCOL, PADN)  # dev [c, n] f16
        outs.append(a[order, :PER].T.astype(np.float32) * scale[None, :])
    return np.concatenate(outs, axis=0), res


def kernel(cos_theta, phi):
    out, _ = _run(cos_theta, phi)
    return out
